# revision 1
# baseline (speedup 1.0000x reference)
"""Trainium2 Bass kernel for nn_EnsembleTransitionModel.

Sharding: model-parallel (expert-parallel). M=8 ensemble members across 8
NeuronCores; each core runs one full MLP over the whole batch. Inputs are
replicated, per-model weights are sharded.

Device layout: activations are kept feature-major (x^T: [features, batch])
so every weight matrix is used directly as the matmul stationary operand
lhsT=[K_in, M_out] without any transposes. BatchNorm (eval) is folded into
a per-feature affine (scale, bias) applied by one scalar-engine Relu
activation straight out of PSUM. The residual z_last (= x rows 1536:1920)
rides a separate fp32 path so the dominant output term stays full precision.

Matmul dtype modes:
  bf16 — weights/activations bf16: LDWEIGHTS is a separate instruction the
         PE pulls ahead of in-flight matmuls, hiding the weight load.
  f32r — rounded fp32 (full-rate 1 cycle/row) but self-loading matmuls pay
         the ~85-cycle weight load serially.
"""

import os
import sys

import numpy as np

for _p in ("/opt/trn_rl_repo", "/root/.axon_site/_ro/trn_rl_repo"):
    if os.path.isdir(_p) and _p not in sys.path:
        sys.path.insert(0, _p)

M = 8
B = 16384
HIST = 5
L = 384
A = 1
HID = 512
NHL = 2
DIN = L * HIST + A * HIST  # 1925
EPS = 1e-5

DT_MODE = "bf16"  # "bf16" | "f32r"

NCH = 512  # batch columns per chunk (= max fp32 moving dim = 1 PSUM bank)
KT1 = 16  # x^T padded to 2048 rows; device uses k-tiles 0..14 (z part) only:
KTZ = 15  # the 5 a_hist rows (a rank-5 term) are precomputed on host as ya
DINP = KT1 * 128
HT = HID // 128  # 4 hidden feature tiles
LT = L // 128  # 3 output feature tiles
ZROW0 = (HIST - 1) * L  # 1536: first row of z_last within x^T

# vecs columns: [b1 (4) | s0 (4) | c0 (4) | s1 (4) | c1 (4) | b3 (3)]
COL_B1 = 0
COL_S = lambda l: 4 + 8 * l
COL_C = lambda l: 8 + 8 * l
COL_B3 = 4 + 8 * NHL
NVEC = COL_B3 + LT


def build_bass(batch=B, dt_mode=DT_MODE):
    import concourse.bacc as bacc
    import concourse.tile as tile
    from concourse import mybir

    f32 = mybir.dt.float32
    mdt = mybir.dt.bfloat16 if dt_mode == "bf16" else mybir.dt.float32r
    Relu = mybir.ActivationFunctionType.Relu
    add = mybir.AluOpType.add

    widths = [NCH] * (batch // NCH)
    assert sum(widths) == batch

    nc = bacc.Bacc("TRN2", target_bir_lowering=False)
    xT = nc.declare_dram_parameter("xT", [DINP, batch], mdt, isOutput=False)
    zT = nc.declare_dram_parameter("zT", [L, batch], f32, isOutput=False)
    ya = nc.declare_dram_parameter("ya", [128, HT, batch], mdt, isOutput=False)
    w1 = nc.declare_dram_parameter("w1", [128, KT1, HID], mdt, isOutput=False)
    wh = nc.declare_dram_parameter("wh", [128, NHL, HT, HT, 128], mdt, isOutput=False)
    w3 = nc.declare_dram_parameter("w3", [128, HT, L], mdt, isOutput=False)
    vecs = nc.declare_dram_parameter("vecs", [128, NVEC], f32, isOutput=False)
    outT = nc.declare_dram_parameter("outT", [L, batch], f32, isOutput=True)

    with tile.TileContext(nc) as tc:
        with (
            tc.tile_pool(name="wt", bufs=1) as wpool,
            tc.tile_pool(name="x", bufs=2) as xpool,
            tc.tile_pool(name="z", bufs=2) as zpool,
            tc.tile_pool(name="h", bufs=3) as hpool,
            tc.tile_pool(name="o", bufs=3) as opool,
            tc.tile_pool(name="ps1", bufs=5, space="PSUM") as ps1pool,
            tc.tile_pool(name="psh", bufs=3, space="PSUM") as pshpool,
        ):
            # per-k-tile weight tiles so the first matmul only waits on its
            # own 256KB slice, not the whole 4MB preload
            w1_sb = []
            for kt in range(KTZ):
                t = wpool.tile([128, HID], mdt, tag=f"w1_{kt}")
                nc.sync.dma_start(out=t[:], in_=w1[:, kt, :])
                w1_sb.append(t)
            # allocate now, DMA after chunk-0's x tiles are queued so the
            # first layer-1 matmuls aren't stuck behind these preloads
            wh_sb = wpool.tile([128, NHL, HT, HT, 128], mdt, tag="wh")
            w3_sb = wpool.tile([128, HT, L], mdt, tag="w3")
            v_sb = wpool.tile([128, NVEC], f32, tag="vecs")

            b0 = 0
            for c, w_c in enumerate(widths):
                # chunk 0: x first (it gates the first matmul group; ya/z are
                # not read until the group completes). Steady state: small
                # ya/z streams first — the L1 psum recycle waits on the ya
                # add, and the out stage on z — don't queue them last.
                def emit_yaz():
                    ya_t = zpool.tile([128, HT, w_c], mdt, tag="ya")
                    nc.sync.dma_start(out=ya_t[:], in_=ya[:, :, b0 : b0 + w_c])
                    zts = []
                    for lt in range(LT):
                        zt = zpool.tile([128, w_c], f32, tag=f"z{lt}")
                        nc.sync.dma_start(
                            out=zt[:],
                            in_=zT[lt * 128 : (lt + 1) * 128, b0 : b0 + w_c],
                        )
                        zts.append(zt)
                    return [ya_t[:, ht, :] for ht in range(HT)], zts

                def emit_x():
                    xts = []
                    for kt in range(KTZ):
                        xt = xpool.tile([128, w_c], mdt, tag=f"x{kt}")
                        nc.sync.dma_start(
                            out=xt[:],
                            in_=xT[kt * 128 : (kt + 1) * 128, b0 : b0 + w_c],
                        )
                        xts.append(xt)
                    return xts

                if c == 0:
                    xts = emit_x()
                    yas, zts = emit_yaz()
                else:
                    yas, zts = emit_yaz()
                    xts = emit_x()

                if c == 0:
                    nc.sync.dma_start(out=wh_sb[:], in_=wh[:])
                    nc.sync.dma_start(out=w3_sb[:], in_=w3[:])
                    nc.sync.dma_start(out=v_sb[:], in_=vecs[:])

                # ---- layer 1: h1 = relu(W1^T x + b1), [512, NCH] ----
                h1 = []
                for ht in range(HT):
                    ps = ps1pool.tile([128, w_c], f32, tag="ps1")
                    for kt in range(KTZ):
                        nc.tensor.matmul(
                            ps[:],
                            w1_sb[kt][:, ht * 128 : (ht + 1) * 128],
                            xts[kt][:],
                            start=(kt == 0),
                            stop=(kt == KTZ - 1),
                        )
                    tsb = hpool.tile([128, w_c], f32, tag=f"t1_{ht}")
                    nc.vector.tensor_tensor(tsb[:], ps[:], yas[ht], add)
                    hsb = hpool.tile([128, w_c], mdt, tag=f"h1_{ht}")
                    nc.scalar.activation(
                        hsb[:], tsb[:], Relu, bias=v_sb[:, COL_B1 + ht : COL_B1 + ht + 1]
                    )
                    h1.append(hsb)

                # ---- hidden layers: h = relu((h @ Wh[l]) * s_l + c_l) ----
                hin = h1
                for l in range(NHL):
                    hout = []
                    for mt in range(HT):
                        ps = pshpool.tile([128, w_c], f32, tag="ps2")
                        for kt in range(HT):
                            nc.tensor.matmul(
                                ps[:],
                                wh_sb[:, l, kt, mt, :],
                                hin[kt][:],
                                start=(kt == 0),
                                stop=(kt == HT - 1),
                            )
                        hsb = hpool.tile([128, w_c], mdt, tag=f"h{l + 2}_{mt}")
                        nc.scalar.activation(
                            hsb[:],
                            ps[:],
                            Relu,
                            bias=v_sb[:, COL_C(l) + mt : COL_C(l) + mt + 1],
                            scale=v_sb[:, COL_S(l) + mt : COL_S(l) + mt + 1],
                        )
                        hout.append(hsb)
                    hin = hout

                # ---- out: delta^T = W3^T h + b3; out = delta^T + zlast^T ----
                for lt in range(LT):
                    ps = ps1pool.tile([128, w_c], f32, tag="ps1")
                    for kt in range(HT):
                        nc.tensor.matmul(
                            ps[:],
                            w3_sb[:, kt, lt * 128 : (lt + 1) * 128],
                            hin[kt][:],
                            start=(kt == 0),
                            stop=(kt == HT - 1),
                        )
                    ot = opool.tile([128, w_c], f32, tag=f"o{lt}")
                    nc.vector.tensor_tensor(ot[:], ps[:], zts[lt][:], add)
                    nc.sync.dma_start(
                        out=outT[lt * 128 : (lt + 1) * 128, b0 : b0 + w_c], in_=ot[:]
                    )
                b0 += w_c
    nc.compile()
    return nc


def _mdt_np(dt_mode):
    if dt_mode == "bf16":
        import ml_dtypes

        return ml_dtypes.bfloat16
    return np.float32


def prep_core_inputs(
    z_hist, a_hist, W1, b1, Wh, bh, gamma, beta, rmean, rvar, W3, b3, dt_mode=DT_MODE
):
    """Host-side shard prep: returns per-model input dicts (xT/zT shared)."""
    mnp = _mdt_np(dt_mode)
    batch = z_hist.shape[0]
    x = np.concatenate(
        [z_hist.reshape(batch, -1), a_hist.reshape(batch, -1)], axis=1
    ).astype(np.float32)
    xT = np.zeros((DINP, batch), mnp)
    xT[:DIN] = x.T.astype(mnp)
    a_flat = x[:, KTZ * 128 :]  # [batch, 5]
    z_lastT = np.ascontiguousarray(x.T[ZROW0 : ZROW0 + L]).astype(np.float32)

    rstd = 1.0 / np.sqrt(rvar.astype(np.float64) + EPS)  # [NHL, M, HID]
    s_aff = (gamma * rstd).astype(np.float32)
    c_aff = ((bh - rmean) * gamma * rstd + beta).astype(np.float32)

    in_maps = []
    for m in range(M):
        w1p = np.zeros((DINP, HID), np.float32)
        w1p[:DIN] = W1[m]
        w1h = np.ascontiguousarray(
            w1p.reshape(KT1, 128, HID).transpose(1, 0, 2)
        ).astype(mnp)  # [128, KT1, HID]

        whh = np.ascontiguousarray(
            Wh[:, m].reshape(NHL, HT, 128, HT, 128).transpose(2, 0, 1, 3, 4)
        ).astype(mnp)  # [128, NHL, kt, mt, 128]

        w3h = np.ascontiguousarray(W3[m].reshape(HT, 128, L).transpose(1, 0, 2)).astype(
            mnp
        )  # [128, HT, L]

        vecs = np.zeros((128, NVEC), np.float32)
        vecs[:, COL_B1 : COL_B1 + HT] = b1[m].reshape(HT, 128).T
        for l in range(NHL):
            vecs[:, COL_S(l) : COL_S(l) + HT] = s_aff[l, m].reshape(HT, 128).T
            vecs[:, COL_C(l) : COL_C(l) + HT] = c_aff[l, m].reshape(HT, 128).T
        vecs[:, COL_B3 : COL_B3 + LT] = b3[m].reshape(LT, 128).T

        zT = z_lastT + b3[m][:, None]  # fold b3 into the residual stream
        y_a = (a_flat @ W1[m][KTZ * 128 :]).T  # [HID, batch] fp32
        yah = np.ascontiguousarray(
            y_a.reshape(HT, 128, batch).transpose(1, 0, 2)
        ).astype(mnp)  # [128, HT, batch]
        in_maps.append(
            {"xT": xT, "zT": zT, "ya": yah, "w1": w1h, "wh": whh, "w3": w3h,
             "vecs": vecs}
        )
    return in_maps


def _reset_device():
    """Clear any exec-unit wedge a previous (profiled) session left behind."""
    try:
        import ctypes

        import jax

        jax.devices()
        lib = ctypes.CDLL("/opt/axon/libaxon_pjrt.so")
        if hasattr(lib, "axon_reset"):
            lib.axon_reset.restype = ctypes.c_int64
            lib.axon_reset()
    except Exception:
        pass


def kernel(**inputs):
    inputs = {k: np.asarray(v) for k, v in inputs.items()}
    in_maps = prep_core_inputs(**inputs)
    nc = build_bass(B)

    from concourse import bass_utils

    _reset_device()
    res = bass_utils.run_bass_kernel_spmd(nc, in_maps, core_ids=list(range(M)))
    out = np.stack(
        [np.ascontiguousarray(res.results[m]["outT"].T) for m in range(M)]
    )  # [M, B, L]
    return out.astype(np.float32)



# revision 4
# speedup vs baseline: 1.5822x; 1.5822x over previous
"""Trainium2 Bass kernel for nn_EnsembleTransitionModel — fp8 DoubleRow edition.

Sharding: model-parallel (expert-parallel). M=8 ensemble members across 8
NeuronCores; each core runs one full MLP over the whole batch. Inputs are
replicated, per-model weights are sharded.

All four matmuls run in fp8e4 (e4m3) with MatmulPerfMode.DoubleRow: the PE
contracts TWO 128-row k-tiles per pass (lhsT [128,2,128], rhs [128,2,N]),
doubling tensor throughput vs bf16. Weights are pre-scaled by SW=64 on the
host so their 0.02-sigma values sit in e4m3's normal range; the inverse
scale folds into the scalar-engine activation (scale operand) for L1/hidden
layers, and into a host-side divide for the output layer (the residual
stream z+b3 is pre-scaled by 64 on host, so out_dram = 64*(delta+z+b3) and
the host divides once after gather).

The 5 a_hist rows ride in k-tile 15 of the padded 2048-row x (no separate
ya stream). x and z streams are chunk-blocked in DRAM so each per-chunk DMA
is one fully contiguous 8KB/partition (x) / 3KB/partition (z) transfer.
Output is written bf16 (error lands on 64*z, ~0.1% — well inside the gate).
"""

import os
import sys

import numpy as np

for _p in ("/opt/trn_rl_repo", "/root/.axon_site/_ro/trn_rl_repo"):
    if os.path.isdir(_p) and _p not in sys.path:
        sys.path.insert(0, _p)

M = 8
B = 16384
HIST = 5
L = 384
A = 1
HID = 512
NHL = 2
DIN = L * HIST + A * HIST  # 1925
EPS = 1e-5

NCH = 512  # batch columns per chunk (= 1 PSUM bank of fp32)
KT = 16  # x^T padded to 2048 rows (1925 real: z rows 0..1919, a rows 1920..1924)
NP1 = KT // 2  # 8 DoubleRow k-tile pairs for layer 1
HT = HID // 128  # 4 hidden feature tiles
HPAIR = HT // 2  # 2 DoubleRow pairs
LT = L // 128  # 3 output feature tiles
ZROW0 = (HIST - 1) * L  # 1536: first row of z_last within x^T
SW = 64.0  # weight pre-scale (power of 2) to center fp8e4m3 range

# vecs columns: [b1 (4) | inv64 (1) | s0 (4) | c0 (4) | s1 (4) | c1 (4)]
COL_B1 = 0
COL_INV = 4
COL_S = lambda l: 5 + 8 * l
COL_C = lambda l: 9 + 8 * l
NVEC = 5 + 8 * NHL


def build_bass(batch=B):
    import concourse.bacc as bacc
    import concourse.tile as tile
    from concourse import mybir

    f32 = mybir.dt.float32
    bf16 = mybir.dt.bfloat16
    fp8 = mybir.dt.float8e4
    DR = mybir.MatmulPerfMode.DoubleRow
    Relu = mybir.ActivationFunctionType.Relu
    add = mybir.AluOpType.add

    nch = batch // NCH
    assert nch * NCH == batch

    nc = bacc.Bacc("TRN2", target_bir_lowering=False)
    x8 = nc.declare_dram_parameter("x8", [nch, 128, KT, NCH], fp8, isOutput=False)
    z64 = nc.declare_dram_parameter("z64", [nch, 128, LT, NCH], bf16, isOutput=False)
    w1 = nc.declare_dram_parameter("w1", [128, KT, HID], fp8, isOutput=False)
    wh = nc.declare_dram_parameter(
        "wh", [128, NHL, HPAIR, 2, HT, 128], fp8, isOutput=False
    )
    w3 = nc.declare_dram_parameter("w3", [128, HPAIR, 2, L], fp8, isOutput=False)
    vecs = nc.declare_dram_parameter("vecs", [128, NVEC], f32, isOutput=False)
    outT = nc.declare_dram_parameter("outT", [L, batch], bf16, isOutput=True)

    with tile.TileContext(nc) as tc:
        with (
            tc.tile_pool(name="wt", bufs=1) as wpool,
            tc.tile_pool(name="x", bufs=2) as xpool,
            tc.tile_pool(name="z", bufs=2) as zpool,
            tc.tile_pool(name="h", bufs=3) as hpool,
            tc.tile_pool(name="o", bufs=3) as opool,
            tc.tile_pool(name="ps1", bufs=5, space="PSUM") as ps1pool,
            tc.tile_pool(name="psh", bufs=3, space="PSUM") as pshpool,
        ):
            # w1 DMA'd per pair so the first matmul only waits on its own slice
            w1_sb = wpool.tile([128, KT, HID], fp8, tag="w1")
            for pr in range(NP1):
                nc.sync.dma_start(
                    out=w1_sb[:, 2 * pr : 2 * pr + 2, :],
                    in_=w1[:, 2 * pr : 2 * pr + 2, :],
                )
            # allocate now, DMA after chunk-0's x tile is queued so the first
            # layer-1 matmuls aren't stuck behind these preloads
            wh_sb = wpool.tile([128, NHL, HPAIR, 2, HT, 128], fp8, tag="wh")
            w3_sb = wpool.tile([128, HPAIR, 2, L], fp8, tag="w3")
            v_sb = wpool.tile([128, NVEC], f32, tag="vecs")

            b0 = 0
            for c in range(nch):
                def emit_x():
                    xt = xpool.tile([128, KT, NCH], fp8, tag="x")
                    nc.sync.dma_start(out=xt[:], in_=x8[c, :, :, :])
                    return xt

                def emit_z():
                    zt = zpool.tile([128, LT, NCH], bf16, tag="z")
                    nc.sync.dma_start(out=zt[:], in_=z64[c, :, :, :])
                    return zt

                # chunk 0: x first (it gates the first matmul group). Steady
                # state: small z stream first so the out stage isn't starved.
                if c == 0:
                    xt = emit_x()
                    zt = emit_z()
                    nc.sync.dma_start(out=v_sb[:], in_=vecs[:])
                    nc.sync.dma_start(out=wh_sb[:], in_=wh[:])
                    nc.sync.dma_start(out=w3_sb[:], in_=w3[:])
                else:
                    zt = emit_z()
                    xt = emit_x()

                # ---- layer 1: h1 = relu((SW*W1^T x)/SW + b1), fp8 out ----
                h1 = [
                    hpool.tile([128, 2, NCH], fp8, tag=f"h1p{j}", name=f"h1p{j}")
                    for j in range(HPAIR)
                ]
                for ht in range(HT):
                    ps = ps1pool.tile([128, NCH], f32, tag="ps1")
                    for pr in range(NP1):
                        nc.tensor.matmul(
                            ps[:],
                            w1_sb[:, 2 * pr : 2 * pr + 2, ht * 128 : (ht + 1) * 128],
                            xt[:, 2 * pr : 2 * pr + 2, :],
                            start=(pr == 0),
                            stop=(pr == NP1 - 1),
                            perf_mode=DR,
                        )
                    nc.scalar.activation(
                        h1[ht // 2][:, ht % 2, :],
                        ps[:],
                        Relu,
                        bias=v_sb[:, COL_B1 + ht : COL_B1 + ht + 1],
                        scale=v_sb[:, COL_INV : COL_INV + 1],
                    )

                # ---- hidden: h = relu((SW*Wh^T h)*(s_aff/SW) + c_aff) ----
                hin = h1
                for l in range(NHL):
                    hout = [
                        hpool.tile(
                            [128, 2, NCH], fp8, tag=f"h{l + 2}p{j}", name=f"h{l + 2}p{j}"
                        )
                        for j in range(HPAIR)
                    ]
                    for mt in range(HT):
                        ps = pshpool.tile([128, NCH], f32, tag="ps2")
                        for pr in range(HPAIR):
                            nc.tensor.matmul(
                                ps[:],
                                wh_sb[:, l, pr, :, mt, :],
                                hin[pr][:],
                                start=(pr == 0),
                                stop=(pr == HPAIR - 1),
                                perf_mode=DR,
                            )
                        nc.scalar.activation(
                            hout[mt // 2][:, mt % 2, :],
                            ps[:],
                            Relu,
                            bias=v_sb[:, COL_C(l) + mt : COL_C(l) + mt + 1],
                            scale=v_sb[:, COL_S(l) + mt : COL_S(l) + mt + 1],
                        )
                    hin = hout

                # ---- out: out64 = SW*W3^T h + SW*(z+b3); host divides by SW ----
                for lt in range(LT):
                    ps = ps1pool.tile([128, NCH], f32, tag="ps1")
                    for pr in range(HPAIR):
                        nc.tensor.matmul(
                            ps[:],
                            w3_sb[:, pr, :, lt * 128 : (lt + 1) * 128],
                            hin[pr][:],
                            start=(pr == 0),
                            stop=(pr == HPAIR - 1),
                            perf_mode=DR,
                        )
                    ot = opool.tile([128, NCH], bf16, tag=f"o{lt}")
                    nc.vector.tensor_tensor(ot[:], ps[:], zt[:, lt, :], add)
                    nc.sync.dma_start(
                        out=outT[lt * 128 : (lt + 1) * 128, b0 : b0 + NCH], in_=ot[:]
                    )
                b0 += NCH
    nc.compile()
    return nc


def prep_core_inputs(
    z_hist, a_hist, W1, b1, Wh, bh, gamma, beta, rmean, rvar, W3, b3
):
    """Host-side shard prep: returns per-model input dicts (x8 shared)."""
    import ml_dtypes

    fp8 = ml_dtypes.float8_e4m3
    bf16 = ml_dtypes.bfloat16
    batch = z_hist.shape[0]
    nch = batch // NCH
    x = np.concatenate(
        [z_hist.reshape(batch, -1), a_hist.reshape(batch, -1)], axis=1
    ).astype(np.float32)
    xpadT = np.zeros((KT * 128, batch), np.float32)
    xpadT[:DIN] = x.T
    x8 = np.ascontiguousarray(
        xpadT.reshape(KT, 128, nch, NCH).transpose(2, 1, 0, 3)
    ).astype(fp8)
    z_lastT = xpadT[ZROW0 : ZROW0 + L]  # [L, batch] f32

    rstd = 1.0 / np.sqrt(rvar.astype(np.float64) + EPS)  # [NHL, M, HID]
    s_aff = (gamma * rstd).astype(np.float32)
    c_aff = ((bh - rmean) * gamma * rstd + beta).astype(np.float32)

    in_maps = []
    for m in range(M):
        w1p = np.zeros((KT * 128, HID), np.float32)
        w1p[:DIN] = W1[m] * SW
        w1h = np.ascontiguousarray(
            w1p.reshape(KT, 128, HID).transpose(1, 0, 2)
        ).astype(fp8)  # [128, KT, HID]

        whh = np.ascontiguousarray(
            (Wh[:, m] * SW)
            .reshape(NHL, HPAIR, 2, 128, HT, 128)
            .transpose(3, 0, 1, 2, 4, 5)
        ).astype(fp8)  # [128, NHL, pr, i, mt, 128]

        w3h = np.ascontiguousarray(
            (W3[m] * SW).reshape(HPAIR, 2, 128, L).transpose(2, 0, 1, 3)
        ).astype(fp8)  # [128, pr, i, L]

        vecs = np.zeros((128, NVEC), np.float32)
        vecs[:, COL_B1 : COL_B1 + HT] = b1[m].reshape(HT, 128).T
        vecs[:, COL_INV] = 1.0 / SW
        for l in range(NHL):
            vecs[:, COL_S(l) : COL_S(l) + HT] = (
                s_aff[l, m].reshape(HT, 128).T / SW
            )
            vecs[:, COL_C(l) : COL_C(l) + HT] = c_aff[l, m].reshape(HT, 128).T

        z64 = np.ascontiguousarray(
            ((z_lastT + b3[m][:, None]) * SW)
            .reshape(LT, 128, nch, NCH)
            .transpose(2, 1, 0, 3)
        ).astype(bf16)  # [nch, 128, LT, NCH]

        in_maps.append(
            {"x8": x8, "z64": z64, "w1": w1h, "wh": whh, "w3": w3h, "vecs": vecs}
        )
    return in_maps


def postprocess(results):
    """[M dicts with outT [L, batch] bf16 * SW] -> [M, batch, L] f32."""
    return np.stack(
        [
            np.ascontiguousarray(results[m]["outT"].T).astype(np.float32) / SW
            for m in range(M)
        ]
    )


def _reset_device():
    """Clear any exec-unit wedge a previous (profiled) session left behind."""
    try:
        import ctypes

        import jax

        jax.devices()
        lib = ctypes.CDLL("/opt/axon/libaxon_pjrt.so")
        if hasattr(lib, "axon_reset"):
            lib.axon_reset.restype = ctypes.c_int64
            lib.axon_reset()
    except Exception:
        pass


def kernel(**inputs):
    inputs = {k: np.asarray(v) for k, v in inputs.items()}
    in_maps = prep_core_inputs(**inputs)
    nc = build_bass(B)

    from concourse import bass_utils

    _reset_device()
    res = bass_utils.run_bass_kernel_spmd(nc, in_maps, core_ids=list(range(M)))
    return postprocess(res.results)


# revision 5
# speedup vs baseline: 1.9348x; 1.2229x over previous
"""Trainium2 Bass kernel for nn_EnsembleTransitionModel — fp8 DoubleRow, 4-stage
software-pipelined edition.

Sharding: model-parallel. M=8 ensemble members across 8 NeuronCores; each core
runs one full MLP over the whole batch. Inputs replicated, weights sharded.

All matmuls are fp8e4 (e4m3) with MatmulPerfMode.DoubleRow (two 128-row
k-tiles per pass, 2x bf16 throughput). The PE instruction stream is software
pipelined four stages deep --

    iter i:  L1(i) | hidden0(i-1) | hidden1(i-2) | out(i-3)

-- so every stage's input activations were produced a full iteration (~13us)
earlier and the PE never waits on the Act/DVE engines (whose per-op latency is
~0.7us on [128,512] tiles). That also keeps the PE pstate clock pinned high.

Numerics: per-layer power-of-2 scale chain. Weights are host-scaled so their
fp8 values sit near sigma~1; each layer's psum carries gamma_l, activations
store alpha_l*h_l in fp8, and the ratio alpha_l/gamma_l is applied as an
immediate scale in the activation op. BatchNorm (eval) scale folds into the
next weight matrix's columns on host; BN bias rides the Act-engine bias
operand; L1's b1 rides a constant-1.0 row in x's padding (row 1925) so the
L1 relu needs no bias and runs on the DVE as one tensor_scalar (mult, max).
The output stage writes 8192*(delta+z+b3) in bf16; the host divides once.
"""

import os
import sys

import numpy as np

for _p in ("/opt/trn_rl_repo", "/root/.axon_site/_ro/trn_rl_repo"):
    if os.path.isdir(_p) and _p not in sys.path:
        sys.path.insert(0, _p)

M = 8
B = 16384
HIST = 5
L = 384
A = 1
HID = 512
NHL = 2
DIN = L * HIST + A * HIST  # 1925
EPS = 1e-5

NCH = 512  # batch columns per chunk (= 1 PSUM bank of fp32)
KT = 16  # x^T padded to 2048 rows (1925 real + ones-row 1925 for b1)
NP1 = KT // 2  # 8 DoubleRow k-tile pairs for layer 1
HT = HID // 128  # 4 hidden feature tiles
HPAIR = HT // 2  # 2 DoubleRow pairs
LT = L // 128  # 3 output feature tiles
ZROW0 = (HIST - 1) * L  # 1536: first row of z_last within x^T
ONEROW = DIN  # 1925: constant-1.0 row carrying b1 into the L1 psum

# per-layer scale chain: psum_l = G_l * (W_l^T h_{l-1}); stored h_l = A_l * h_l
G1, A1 = 64.0, 16.0
G2, A2 = 1024.0, 64.0
G3, A3 = 4096.0, 128.0
G4 = 8192.0  # output psum & residual scale; host divides by G4
SC_L1 = A1 / G1  # immediate scale in the L1 DVE relu
SC_H = (A2 / G2, A3 / G3)  # immediate scales in the hidden Act relus

# vecs columns: [A2*c0 (4) | A3*c1 (4)]
COL_C = lambda l: 4 * l
NVEC = 4 * NHL


def build_bass(batch=B):
    import concourse.bacc as bacc
    import concourse.tile as tile
    from concourse import mybir

    f32 = mybir.dt.float32
    bf16 = mybir.dt.bfloat16
    fp8 = mybir.dt.float8e4
    DR = mybir.MatmulPerfMode.DoubleRow
    Relu = mybir.ActivationFunctionType.Relu
    add = mybir.AluOpType.add
    mult = mybir.AluOpType.mult
    amax = mybir.AluOpType.max

    nch = batch // NCH
    assert nch * NCH == batch

    nc = bacc.Bacc("TRN2", target_bir_lowering=False)
    x8 = nc.declare_dram_parameter("x8", [nch, 128, KT, NCH], fp8, isOutput=False)
    zg = nc.declare_dram_parameter("zg", [nch, 128, LT, NCH], bf16, isOutput=False)
    w1 = nc.declare_dram_parameter("w1", [128, KT, HID], fp8, isOutput=False)
    wh = nc.declare_dram_parameter(
        "wh", [128, NHL, HPAIR, 2, HT, 128], fp8, isOutput=False
    )
    w3 = nc.declare_dram_parameter("w3", [128, HPAIR, 2, L], fp8, isOutput=False)
    vecs = nc.declare_dram_parameter("vecs", [128, NVEC], f32, isOutput=False)
    outT = nc.declare_dram_parameter("outT", [L, batch], bf16, isOutput=True)

    with tile.TileContext(nc) as tc:
        with (
            tc.tile_pool(name="wt", bufs=1) as wpool,
            tc.tile_pool(name="x", bufs=3) as xpool,
            tc.tile_pool(name="z", bufs=6) as zpool,
            tc.tile_pool(name="h", bufs=3) as hpool,
            tc.tile_pool(name="o", bufs=3) as opool,
            tc.tile_pool(name="ps1", bufs=4, space="PSUM") as ps1pool,
            tc.tile_pool(name="psh", bufs=4, space="PSUM") as pshpool,
        ):
            # w1 DMA'd per pair so the first matmul only waits on its own slice
            w1_sb = wpool.tile([128, KT, HID], fp8, tag="w1")
            for pr in range(NP1):
                nc.sync.dma_start(
                    out=w1_sb[:, 2 * pr : 2 * pr + 2, :],
                    in_=w1[:, 2 * pr : 2 * pr + 2, :],
                )
            wh_sb = wpool.tile([128, NHL, HPAIR, 2, HT, 128], fp8, tag="wh")
            w3_sb = wpool.tile([128, HPAIR, 2, L], fp8, tag="w3")
            v_sb = wpool.tile([128, NVEC], f32, tag="vecs")

            xts, zts, h1s, h2s, h3s = {}, {}, {}, {}, {}

            def dma_x(c):
                xt = xpool.tile([128, KT, NCH], fp8, tag="x", name="xt")
                nc.sync.dma_start(out=xt[:], in_=x8[c, :, :, :])
                xts[c] = xt

            def dma_z(c):
                zt = zpool.tile([128, LT, NCH], bf16, tag="z", name="zt")
                nc.sync.dma_start(out=zt[:], in_=zg[c, :, :, :])
                zts[c] = zt

            def stage_l1(c):
                xt = xts.pop(c)
                h1p = [
                    hpool.tile([128, 2, NCH], fp8, tag=f"h1p{j}", name=f"h1p{j}")
                    for j in range(HPAIR)
                ]
                for ht in range(HT):
                    ps = ps1pool.tile([128, NCH], f32, tag="ps1", name="ps")
                    for pr in range(NP1):
                        nc.tensor.matmul(
                            ps[:],
                            w1_sb[:, 2 * pr : 2 * pr + 2, ht * 128 : (ht + 1) * 128],
                            xt[:, 2 * pr : 2 * pr + 2, :],
                            start=(pr == 0),
                            stop=(pr == NP1 - 1),
                            perf_mode=DR,
                        )
                    # relu on the DVE: h1 = max(SC_L1 * psum, 0); b1 already in
                    # psum via the ones-row
                    nc.vector.tensor_scalar(
                        h1p[ht // 2][:, ht % 2, :], ps[:], SC_L1, 0.0, mult, amax
                    )
                h1s[c] = h1p

            def stage_hidden(l, src, dst, c):
                hin = src.pop(c)
                hout = [
                    hpool.tile(
                        [128, 2, NCH], fp8, tag=f"h{l + 2}p{j}", name=f"h{l + 2}p{j}"
                    )
                    for j in range(HPAIR)
                ]
                for mt in range(HT):
                    ps = pshpool.tile([128, NCH], f32, tag="ps2", name="ps")
                    for pr in range(HPAIR):
                        nc.tensor.matmul(
                            ps[:],
                            wh_sb[:, l, pr, :, mt, :],
                            hin[pr][:],
                            start=(pr == 0),
                            stop=(pr == HPAIR - 1),
                            perf_mode=DR,
                        )
                    nc.scalar.activation(
                        hout[mt // 2][:, mt % 2, :],
                        ps[:],
                        Relu,
                        bias=v_sb[:, COL_C(l) + mt : COL_C(l) + mt + 1],
                        scale=SC_H[l],
                    )
                dst[c] = hout

            def stage_out(c):
                hin = h3s.pop(c)
                zt = zts.pop(c)
                for lt in range(LT):
                    ps = ps1pool.tile([128, NCH], f32, tag="ps1", name="ps")
                    for pr in range(HPAIR):
                        nc.tensor.matmul(
                            ps[:],
                            w3_sb[:, pr, :, lt * 128 : (lt + 1) * 128],
                            hin[pr][:],
                            start=(pr == 0),
                            stop=(pr == HPAIR - 1),
                            perf_mode=DR,
                        )
                    ot = opool.tile([128, NCH], bf16, tag=f"o{lt}", name=f"o{lt}")
                    nc.vector.tensor_tensor(ot[:], ps[:], zt[:, lt, :], add)
                    nc.sync.dma_start(
                        out=outT[lt * 128 : (lt + 1) * 128, c * NCH : (c + 1) * NCH],
                        in_=ot[:],
                    )

            dma_x(0)
            for i in range(nch + 3):
                if i + 1 < nch:
                    dma_x(i + 1)
                if i == 0:
                    nc.sync.dma_start(out=v_sb[:], in_=vecs[:])
                    nc.sync.dma_start(out=wh_sb[:], in_=wh[:])
                    nc.sync.dma_start(out=w3_sb[:], in_=w3[:])
                if i < nch:
                    dma_z(i)
                if i < nch:
                    stage_l1(i)
                if 1 <= i < nch + 1:
                    stage_hidden(0, h1s, h2s, i - 1)
                if 2 <= i < nch + 2:
                    stage_hidden(1, h2s, h3s, i - 2)
                if i >= 3:
                    stage_out(i - 3)
    nc.compile()
    return nc


def prep_core_inputs(
    z_hist, a_hist, W1, b1, Wh, bh, gamma, beta, rmean, rvar, W3, b3
):
    """Host-side shard prep: returns per-model input dicts (x8 shared)."""
    import ml_dtypes

    fp8 = ml_dtypes.float8_e4m3
    bf16 = ml_dtypes.bfloat16
    batch = z_hist.shape[0]
    nch = batch // NCH
    x = np.concatenate(
        [z_hist.reshape(batch, -1), a_hist.reshape(batch, -1)], axis=1
    ).astype(np.float32)
    xpadT = np.zeros((KT * 128, batch), np.float32)
    xpadT[:DIN] = x.T
    xpadT[ONEROW] = 1.0  # carries b1 through the L1 matmul
    x8 = np.ascontiguousarray(
        xpadT.reshape(KT, 128, nch, NCH).transpose(2, 1, 0, 3)
    ).astype(fp8)
    z_lastT = xpadT[ZROW0 : ZROW0 + L]  # [L, batch] f32

    rstd = 1.0 / np.sqrt(rvar.astype(np.float64) + EPS)  # [NHL, M, HID]
    s_aff = (gamma * rstd).astype(np.float32)
    c_aff = ((bh - rmean) * gamma * rstd + beta).astype(np.float32)

    in_maps = []
    for m in range(M):
        w1p = np.zeros((KT * 128, HID), np.float32)
        w1p[:DIN] = W1[m] * G1
        w1p[ONEROW] = b1[m] * G1
        w1h = np.ascontiguousarray(
            w1p.reshape(KT, 128, HID).transpose(1, 0, 2)
        ).astype(fp8)  # [128, KT, HID]

        # BN scale of layer l folds into Wh[l]'s columns; weight rows absorb
        # the previous layer's stored-activation scale A_{l-1}
        whs = np.stack(
            [
                Wh[0, m] * s_aff[0, m][None, :] * (G2 / A1),
                Wh[1, m] * s_aff[1, m][None, :] * (G3 / A2),
            ]
        )
        whh = np.ascontiguousarray(
            whs.reshape(NHL, HPAIR, 2, 128, HT, 128).transpose(3, 0, 1, 2, 4, 5)
        ).astype(fp8)  # [128, NHL, pr, i, mt, 128]

        w3h = np.ascontiguousarray(
            (W3[m] * (G4 / A3)).reshape(HPAIR, 2, 128, L).transpose(2, 0, 1, 3)
        ).astype(fp8)  # [128, pr, i, L]

        vecs = np.zeros((128, NVEC), np.float32)
        vecs[:, COL_C(0) : COL_C(0) + HT] = (c_aff[0, m] * A2).reshape(HT, 128).T
        vecs[:, COL_C(1) : COL_C(1) + HT] = (c_aff[1, m] * A3).reshape(HT, 128).T

        zgm = np.ascontiguousarray(
            ((z_lastT + b3[m][:, None]) * G4)
            .reshape(LT, 128, nch, NCH)
            .transpose(2, 1, 0, 3)
        ).astype(bf16)  # [nch, 128, LT, NCH]

        in_maps.append(
            {"x8": x8, "zg": zgm, "w1": w1h, "wh": whh, "w3": w3h, "vecs": vecs}
        )
    return in_maps


def postprocess(results):
    """[M dicts with outT [L, batch] bf16 * G4] -> [M, batch, L] f32."""
    return np.stack(
        [
            np.ascontiguousarray(results[m]["outT"].T).astype(np.float32) / G4
            for m in range(M)
        ]
    )


def _reset_device():
    """Clear any exec-unit wedge a previous (profiled) session left behind."""
    try:
        import ctypes

        import jax

        jax.devices()
        lib = ctypes.CDLL("/opt/axon/libaxon_pjrt.so")
        if hasattr(lib, "axon_reset"):
            lib.axon_reset.restype = ctypes.c_int64
            lib.axon_reset()
    except Exception:
        pass


def kernel(**inputs):
    inputs = {k: np.asarray(v) for k, v in inputs.items()}
    in_maps = prep_core_inputs(**inputs)
    nc = build_bass(B)

    from concourse import bass_utils

    _reset_device()
    res = bass_utils.run_bass_kernel_spmd(nc, in_maps, core_ids=list(range(M)))
    return postprocess(res.results)


# revision 9
# speedup vs baseline: 1.9635x; 1.0149x over previous
"""Trainium2 Bass kernel for nn_EnsembleTransitionModel — fp8 DoubleRow, 4-stage
software-pipelined edition.

Sharding: model-parallel. M=8 ensemble members across 8 NeuronCores; each core
runs one full MLP over the whole batch. Inputs replicated, weights sharded.

All matmuls are fp8e4 (e4m3) with MatmulPerfMode.DoubleRow (two 128-row
k-tiles per pass, 2x bf16 throughput). The PE instruction stream is software
pipelined four stages deep --

    iter i:  L1(i) | hidden0(i-1) | hidden1(i-2) | out(i-3)

-- so every stage's input activations were produced a full iteration (~13us)
earlier and the PE never waits on the Act/DVE engines (whose per-op latency is
~0.7us on [128,512] tiles). That also keeps the PE pstate clock pinned high.

Numerics: per-layer power-of-2 scale chain. Weights are host-scaled so their
fp8 values sit near sigma~1; each layer's psum carries gamma_l, activations
store alpha_l*h_l in fp8, and the ratio alpha_l/gamma_l is applied as an
immediate scale in the activation op. BatchNorm (eval) scale folds into the
next weight matrix's columns on host; BN bias rides the Act-engine bias
operand; L1's b1 rides a constant-1.0 row in x's padding (row 1925) so the
L1 relu needs no bias and runs on the DVE as one tensor_scalar (mult, max).
The output stage writes 8192*(delta+z+b3) in bf16; the host divides once.
"""

import os
import sys

import numpy as np

for _p in ("/opt/trn_rl_repo", "/root/.axon_site/_ro/trn_rl_repo"):
    if os.path.isdir(_p) and _p not in sys.path:
        sys.path.insert(0, _p)

M = 8
B = 16384
HIST = 5
L = 384
A = 1
HID = 512
NHL = 2
DIN = L * HIST + A * HIST  # 1925
EPS = 1e-5

NCH = 512  # batch columns per chunk (= 1 PSUM bank of fp32)
KT = 16  # x^T padded to 2048 rows (1925 real + ones-row 1925 for b1)
NP1 = KT // 2  # 8 DoubleRow k-tile pairs for layer 1
HT = HID // 128  # 4 hidden feature tiles
HPAIR = HT // 2  # 2 DoubleRow pairs
LT = L // 128  # 3 output feature tiles
ZROW0 = (HIST - 1) * L  # 1536: first row of z_last within x^T
ONEROW = DIN  # 1925: constant-1.0 row carrying b1 into the L1 psum

# per-layer scale chain: psum_l = G_l * (W_l^T h_{l-1}); stored h_l = A_l * h_l
G1, A1 = 64.0, 16.0
G2, A2 = 1024.0, 64.0
G3, A3 = 4096.0, 128.0
G4 = 8192.0  # output psum & residual scale; host divides by G4
SC_L1 = A1 / G1  # immediate scale in the L1 DVE relu
SC_H = (A2 / G2, A3 / G3)  # immediate scales in the hidden Act relus

# vecs columns: [A2*c0 (4) | A3*c1 (4)]
COL_C = lambda l: 4 * l
NVEC = 4 * NHL


def build_bass(batch=B):
    import concourse.bacc as bacc
    import concourse.tile as tile
    from concourse import mybir

    f32 = mybir.dt.float32
    bf16 = mybir.dt.bfloat16
    fp8 = mybir.dt.float8e4
    DR = mybir.MatmulPerfMode.DoubleRow
    Relu = mybir.ActivationFunctionType.Relu
    add = mybir.AluOpType.add
    mult = mybir.AluOpType.mult
    amax = mybir.AluOpType.max

    nch = batch // NCH
    assert nch * NCH == batch

    nc = bacc.Bacc("TRN2", target_bir_lowering=False)
    x8 = nc.declare_dram_parameter("x8", [nch, 128, KT, NCH], fp8, isOutput=False)
    zg = nc.declare_dram_parameter("zg", [nch, 128, LT, NCH], bf16, isOutput=False)
    w1 = nc.declare_dram_parameter("w1", [128, KT, HID], fp8, isOutput=False)
    wh = nc.declare_dram_parameter(
        "wh", [128, NHL, HPAIR, 2, HT, 128], fp8, isOutput=False
    )
    w3 = nc.declare_dram_parameter("w3", [128, HPAIR, 2, L], fp8, isOutput=False)
    vecs = nc.declare_dram_parameter("vecs", [128, NVEC], f32, isOutput=False)
    # chunk-blocked so each chunk's output is one fully-contiguous 384KB DMA
    outc = nc.declare_dram_parameter("outc", [nch, 128, LT, NCH], bf16, isOutput=True)

    with tile.TileContext(nc) as tc:
        with (
            tc.tile_pool(name="wt", bufs=1) as wpool,
            tc.tile_pool(name="x", bufs=3) as xpool,
            tc.tile_pool(name="z", bufs=6) as zpool,
            tc.tile_pool(name="h", bufs=3) as hpool,
            tc.tile_pool(name="o", bufs=3) as opool,
            tc.tile_pool(name="ps1", bufs=4, space="PSUM") as ps1pool,
            tc.tile_pool(name="psh", bufs=4, space="PSUM") as pshpool,
        ):
            w1_sb = wpool.tile([128, KT, HID], fp8, tag="w1")
            wh_sb = wpool.tile([128, NHL, HPAIR, 2, HT, 128], fp8, tag="wh")
            w3_sb = wpool.tile([128, HPAIR, 2, L], fp8, tag="w3")
            v_sb = wpool.tile([128, NVEC], f32, tag="vecs")

            xts, zts, h1s, h2s, h3s = {}, {}, {}, {}, {}

            def dma_x(c, per_pair=False):
                xt = xpool.tile([128, KT, NCH], fp8, tag="x", name="xt")
                if per_pair:
                    # chunk 0: interleave w1/x pair slices so the first matmul
                    # group starts after ~256KB instead of the full preload
                    for pr in range(NP1):
                        nc.sync.dma_start(
                            out=w1_sb[:, 2 * pr : 2 * pr + 2, :],
                            in_=w1[:, 2 * pr : 2 * pr + 2, :],
                        )
                        nc.sync.dma_start(
                            out=xt[:, 2 * pr : 2 * pr + 2, :],
                            in_=x8[c, :, 2 * pr : 2 * pr + 2, :],
                        )
                else:
                    nc.sync.dma_start(out=xt[:], in_=x8[c, :, :, :])
                xts[c] = xt

            def dma_z(c):
                zt = zpool.tile([128, LT, NCH], bf16, tag="z", name="zt")
                nc.sync.dma_start(out=zt[:], in_=zg[c, :, :, :])
                zts[c] = zt

            def stage_l1(c):
                xt = xts.pop(c)
                h1p = [
                    hpool.tile([128, 2, NCH], fp8, tag=f"h1p{j}", name=f"h1p{j}")
                    for j in range(HPAIR)
                ]
                for ht in range(HT):
                    ps = ps1pool.tile([128, NCH], f32, tag="ps1", name="ps")
                    for pr in range(NP1):
                        nc.tensor.matmul(
                            ps[:],
                            w1_sb[:, 2 * pr : 2 * pr + 2, ht * 128 : (ht + 1) * 128],
                            xt[:, 2 * pr : 2 * pr + 2, :],
                            start=(pr == 0),
                            stop=(pr == NP1 - 1),
                            perf_mode=DR,
                        )
                    # relu on the DVE: h1 = max(SC_L1 * psum, 0); b1 already in
                    # psum via the ones-row
                    nc.vector.tensor_scalar(
                        h1p[ht // 2][:, ht % 2, :], ps[:], SC_L1, 0.0, mult, amax
                    )
                h1s[c] = h1p

            def stage_hidden(l, src, dst, c):
                hin = src.pop(c)
                hout = [
                    hpool.tile(
                        [128, 2, NCH], fp8, tag=f"h{l + 2}p{j}", name=f"h{l + 2}p{j}"
                    )
                    for j in range(HPAIR)
                ]
                for mt in range(HT):
                    ps = pshpool.tile([128, NCH], f32, tag="ps2", name="ps")
                    for pr in range(HPAIR):
                        nc.tensor.matmul(
                            ps[:],
                            wh_sb[:, l, pr, :, mt, :],
                            hin[pr][:],
                            start=(pr == 0),
                            stop=(pr == HPAIR - 1),
                            perf_mode=DR,
                        )
                    nc.scalar.activation(
                        hout[mt // 2][:, mt % 2, :],
                        ps[:],
                        Relu,
                        bias=v_sb[:, COL_C(l) + mt : COL_C(l) + mt + 1],
                        scale=SC_H[l],
                    )
                dst[c] = hout

            def stage_out(c):
                hin = h3s.pop(c)
                zt = zts.pop(c)
                ot = opool.tile([128, LT, NCH], bf16, tag="ot", name="ot")
                for lt in range(LT):
                    ps = ps1pool.tile([128, NCH], f32, tag="ps1", name="ps")
                    for pr in range(HPAIR):
                        nc.tensor.matmul(
                            ps[:],
                            w3_sb[:, pr, :, lt * 128 : (lt + 1) * 128],
                            hin[pr][:],
                            start=(pr == 0),
                            stop=(pr == HPAIR - 1),
                            perf_mode=DR,
                        )
                    nc.vector.tensor_tensor(ot[:, lt, :], ps[:], zt[:, lt, :], add)
                nc.sync.dma_start(out=outc[c, :, :, :], in_=ot[:])

            dma_x(0, per_pair=True)
            for i in range(nch + 3):
                if i + 1 < nch:
                    dma_x(i + 1)
                if i == 0:
                    nc.sync.dma_start(out=v_sb[:], in_=vecs[:])
                    nc.sync.dma_start(out=wh_sb[:], in_=wh[:])
                    nc.sync.dma_start(out=w3_sb[:], in_=w3[:])
                if i < nch:
                    dma_z(i)
                if i < nch:
                    stage_l1(i)
                if 1 <= i < nch + 1:
                    stage_hidden(0, h1s, h2s, i - 1)
                if 2 <= i < nch + 2:
                    stage_hidden(1, h2s, h3s, i - 2)
                if i >= 3:
                    stage_out(i - 3)
    nc.compile()
    return nc


def prep_core_inputs(
    z_hist, a_hist, W1, b1, Wh, bh, gamma, beta, rmean, rvar, W3, b3
):
    """Host-side shard prep: returns per-model input dicts (x8 shared)."""
    import ml_dtypes

    fp8 = ml_dtypes.float8_e4m3
    bf16 = ml_dtypes.bfloat16
    batch = z_hist.shape[0]
    nch = batch // NCH
    x = np.concatenate(
        [z_hist.reshape(batch, -1), a_hist.reshape(batch, -1)], axis=1
    ).astype(np.float32)
    xpadT = np.zeros((KT * 128, batch), np.float32)
    xpadT[:DIN] = x.T
    xpadT[ONEROW] = 1.0  # carries b1 through the L1 matmul
    x8 = np.ascontiguousarray(
        xpadT.reshape(KT, 128, nch, NCH).transpose(2, 1, 0, 3)
    ).astype(fp8)
    z_lastT = xpadT[ZROW0 : ZROW0 + L]  # [L, batch] f32

    rstd = 1.0 / np.sqrt(rvar.astype(np.float64) + EPS)  # [NHL, M, HID]
    s_aff = (gamma * rstd).astype(np.float32)
    c_aff = ((bh - rmean) * gamma * rstd + beta).astype(np.float32)

    in_maps = []
    for m in range(M):
        w1p = np.zeros((KT * 128, HID), np.float32)
        w1p[:DIN] = W1[m] * G1
        w1p[ONEROW] = b1[m] * G1
        w1h = np.ascontiguousarray(
            w1p.reshape(KT, 128, HID).transpose(1, 0, 2)
        ).astype(fp8)  # [128, KT, HID]

        # BN scale of layer l folds into Wh[l]'s columns; weight rows absorb
        # the previous layer's stored-activation scale A_{l-1}
        whs = np.stack(
            [
                Wh[0, m] * s_aff[0, m][None, :] * (G2 / A1),
                Wh[1, m] * s_aff[1, m][None, :] * (G3 / A2),
            ]
        )
        whh = np.ascontiguousarray(
            whs.reshape(NHL, HPAIR, 2, 128, HT, 128).transpose(3, 0, 1, 2, 4, 5)
        ).astype(fp8)  # [128, NHL, pr, i, mt, 128]

        w3h = np.ascontiguousarray(
            (W3[m] * (G4 / A3)).reshape(HPAIR, 2, 128, L).transpose(2, 0, 1, 3)
        ).astype(fp8)  # [128, pr, i, L]

        vecs = np.zeros((128, NVEC), np.float32)
        vecs[:, COL_C(0) : COL_C(0) + HT] = (c_aff[0, m] * A2).reshape(HT, 128).T
        vecs[:, COL_C(1) : COL_C(1) + HT] = (c_aff[1, m] * A3).reshape(HT, 128).T

        zgm = np.ascontiguousarray(
            ((z_lastT + b3[m][:, None]) * G4)
            .reshape(LT, 128, nch, NCH)
            .transpose(2, 1, 0, 3)
        ).astype(bf16)  # [nch, 128, LT, NCH]

        in_maps.append(
            {"x8": x8, "zg": zgm, "w1": w1h, "wh": whh, "w3": w3h, "vecs": vecs}
        )
    return in_maps


def postprocess(results):
    """[M dicts with outc [nch, 128, LT, NCH] bf16 * G4] -> [M, batch, L] f32."""
    outs = []
    for m in range(M):
        a = results[m]["outc"].astype(np.float32) / G4  # [nch, 128, LT, NCH]
        outs.append(a.transpose(0, 3, 2, 1).reshape(-1, L))  # [batch, L]
    return np.stack(outs)


def _reset_device():
    """Clear any exec-unit wedge a previous (profiled) session left behind."""
    try:
        import ctypes

        import jax

        jax.devices()
        lib = ctypes.CDLL("/opt/axon/libaxon_pjrt.so")
        if hasattr(lib, "axon_reset"):
            lib.axon_reset.restype = ctypes.c_int64
            lib.axon_reset()
    except Exception:
        pass


def kernel(**inputs):
    inputs = {k: np.asarray(v) for k, v in inputs.items()}
    in_maps = prep_core_inputs(**inputs)
    nc = build_bass(B)

    from concourse import bass_utils

    _reset_device()
    res = bass_utils.run_bass_kernel_spmd(nc, in_maps, core_ids=list(range(M)))
    return postprocess(res.results)
